# revision 18
# baseline (speedup 1.0000x reference)
"""Causal single-head attention (B=4, S=4096, D=2048) on 8 trn2 NeuronCores.

Sharding: core = (batch b, query-half h). Query blocks of 128 rows are
interleaved between the two halves ({4j,4j+3} vs {4j+1,4j+2} within each
group of 4) so that both halves execute an identical instruction stream
(SPMD) with balanced causal work. Per core: 8 strips of 256 queries;
strip j attends to keys [0, 512*(j+1)).

v5 = v4 (host-precomputed A = Wq @ Wk^T, no on-device K projection)
+ DMA overhaul. Hardware probing showed each DGE queue sustains only
~47 GB/s for 128-row strided tile loads (~66 GB/s contiguous), so the
~150 MB/core of traffic on 2 queues (~1.6 ms) hid the 0.92 ms of PE
work entirely. v5 therefore:
  - re-packs every DRAM operand on the host into tile-contiguous
    layouts ([.., 128, 512] blocks) so each dma_start is one linear
    256 KB read instead of 128 strided 1 KB rows;
  - spreads traffic over all three DMA-capable queues (sync/SP,
    scalar/Activation, gpsimd/Pool-SWDGE): kg+qs+masks on sync,
    vt+xt on scalar, A+qT+wv+out on gpsimd (~50 MB each);
  - the device writes out in [i, ep, u, 128, 512] tile order and the
    host reassembles (host-side reordering is free).
Compute structure is v4's: q'-projection, scores q'.x^T per 512-key
group with exp + causal co-skips/masks, Z^T = x^T P^T d-major, then
out = Z @ Wv scaled by reciprocal ones-matmul denominators.
"""

import sys

try:
    import concourse  # noqa: F401
except ImportError:
    sys.path.insert(0, "/opt/trn_rl_repo")

import numpy as np
import ml_dtypes

import concourse.bass as bass
import concourse.mybir as mybir
import concourse.tile as tile
from concourse import bacc
from concourse.bass_utils import run_bass_kernel_spmd

B, S, D = 4, 4096, 2048
NQ = S // 2          # queries per core
C = D // 128         # 16 contraction chunks
STRIPS = 8           # strips of 256 queries per core
SQ = NQ // STRIPS    # 256
NG = S // 512        # 8 key groups
SCALE = 1.0 / float(np.sqrt(D))

BF = mybir.dt.bfloat16
F32 = mybir.dt.float32


def _blocks_for_half(h: int) -> list[int]:
    # strip-major order; strip j covers global blocks {4j+0,4j+3} or {4j+1,4j+2}
    off = (0, 3) if h == 0 else (1, 2)
    return [4 * j + o for j in range(STRIPS) for o in off]


def build_nc(variant="full", reps=1):
    nc = bacc.Bacc("TRN2", target_bir_lowering=False, debug=False, num_devices=8)

    # tile-contiguous layouts: trailing [128, 512] blocks are linear in DRAM
    xqT = nc.dram_tensor("xqT", [C, NQ // 512, 128, 512], BF, kind="ExternalInput")
    xkT = nc.dram_tensor("xkT", [C, NG, 128, 512], BF, kind="ExternalInput")
    # x natural layout (keys-major), tiled per (qp, g, kk) for the Z matmul
    xn = nc.dram_tensor("xn", [4, NG, 4, 128, 512], BF, kind="ExternalInput")
    wa = nc.dram_tensor("A", [4, C, 128, 512], BF, kind="ExternalInput")  # Wq @ Wk^T
    wv = nc.dram_tensor("Wv", [D, D], BF, kind="ExternalInput")
    # maskT[512*j + kk, qq]: multiplicative mask for strip j's diagonal key
    # group, key-major (matches the transposed score layout)
    maskT = nc.dram_tensor("maskT", [S, SQ], BF, kind="ExternalInput")
    out = nc.dram_tensor("out", [4, 4, 4, 128, 512], F32, kind="ExternalOutput")

    qT = nc.dram_tensor("qT", [C, NQ // 512, 128, 512], BF, kind="Internal")

    with tile.TileContext(nc) as tc:
        for _rep in range(reps):
            _emit(nc, tc, xqT, xkT, xn, wa, wv, maskT, out, qT, variant)

    nc.compile()
    return nc


def _emit(nc, tc, xqT, xkT, xn, wa, wv, maskT, out, qT, variant="full"):

    if variant == "cast":
        with tc.tile_pool(name="dummy", bufs=1) as dp:
            z = dp.tile([128, 512], F32)
            nc.vector.memset(z[:], 0.0)
            for i in range(4):
                for ep in range(4):
                    for u in range(4):
                        nc.sync.dma_start(out=out.ap()[i, ep, u, :, :], in_=z[:])
        return

    if variant == "dmaonly":
        # Full kernel's load traffic in v5 layouts on the same 3 queues.
        with (
            tc.tile_pool(name="da", bufs=24) as dap,
            tc.tile_pool(name="db", bufs=16) as dbp,
            tc.tile_pool(name="dwv", bufs=4) as dwvp,
            tc.tile_pool(name="dmk", bufs=4) as dmkp,
            tc.tile_pool(name="dz", bufs=2) as dzp,
        ):
            z = dzp.tile([128, 512], BF, name="d_z")
            nc.vector.memset(z[:], 0.0)
            # projection-phase: A (x2 sb) + xqT + qT write
            for sb in range(2):
                for s4 in range(2):
                    for c in range(C):
                        t = dbp.tile([128, 512], BF, name="d_xt")
                        nc.scalar.dma_start(out=t[:], in_=xqT.ap()[c, 2 * sb + s4, :, :])
                for qtr in range(4):
                    for c in range(C):
                        t = dbp.tile([128, 512], BF, name="d_w")
                        nc.gpsimd.dma_start(out=t[:], in_=wa.ap()[qtr, c, :, :])
            for c in range(C):
                for s in range(4):
                    nc.gpsimd.dma_start(out=qT.ap()[c, s, :, :], in_=z[:])
            for c in range(C):
                t = dwvp.tile([128, 2048], BF, name="d_wv")
                nc.gpsimd.dma_start(out=t[:], in_=wv.ap()[128 * c : 128 * (c + 1), :])
            # attention-phase
            zo = dzp.tile([128, 512], F32, name="d_o")
            nc.vector.memset(zo[:], 0.0)
            for i in range(4):
                ng_odd = 2 * i + 2
                for c in range(C):
                    t = dap.tile([128, 512], BF, name="d_qs")
                    nc.sync.dma_start(out=t[:], in_=qT.ap()[c, i, :, :])
                for g in range(ng_odd):
                    for c in range(C):
                        t = dap.tile([128, 512], BF, name="d_kg")
                        nc.sync.dma_start(out=t[:], in_=xkT.ap()[c, g, :, :])
                for kk in range(8):
                    mk = dmkp.tile([128, SQ], BF, name="d_msk")
                    r0 = 512 * i + 128 * (kk % 4)
                    nc.sync.dma_start(out=mk[:], in_=maskT.ap()[r0 : r0 + 128, :])
                for qp in range(4):
                    for g in range(ng_odd):
                        for kk in range(4):
                            vt = dbp.tile([128, 512], BF, name="d_vt")
                            nc.scalar.dma_start(out=vt[:], in_=xn.ap()[qp, g, kk, :, :])
                for ep in range(4):
                    for u in range(4):
                        nc.gpsimd.dma_start(out=out.ap()[i, ep, u, :, :], in_=zo[:])
        return

    # kg/qs pools hoisted: their prefetch DMAs overlap the projections.
    with (
        tc.tile_pool(name="qs", bufs=20) as qsp,
        tc.tile_pool(name="kg", bufs=24) as kgp,
    ):
        # ---- Projection: qT[c, :, s] = (x_q @ A)^T (d-major). A streamed
        # as [128, 512] m-quarter tiles (gpsimd queue); x-tiles on scalar;
        # stationary tile reused across SB seq-tiles so LDWEIGHTS amortizes.
        SB = 2  # seq-tiles per block
        with (
            tc.tile_pool(name="w", bufs=6) as wp,
            tc.tile_pool(name="xt", bufs=36) as xtp,
            tc.tile_pool(name="pps", bufs=8, space="PSUM") as pps,
            tc.tile_pool(name="pcp", bufs=8) as pcp,
        ):
            def project_dmajor(w_dram, xT_dram, n_rows, outT):
                for sb in range(n_rows // 512 // SB):
                    xt = {}
                    for s4 in range(SB):
                        s = SB * sb + s4
                        for c in range(C):
                            t = xtp.tile([128, 512], BF, name="xt")
                            nc.scalar.dma_start(out=t[:], in_=xT_dram.ap()[c, s, :, :])
                            xt[(s4, c)] = t
                    for qtr in range(4):
                        w_q = []
                        for c in range(C):
                            t = wp.tile([128, 512], BF, name=f"wq{c}")
                            nc.gpsimd.dma_start(out=t[:], in_=w_dram.ap()[qtr, c, :, :])
                            w_q.append(t)
                        for mi in range(4):
                            m = 4 * qtr + mi
                            ps = [
                                pps.tile([128, 512], F32, name="pps_t")
                                for _ in range(SB)
                            ]
                            for c in range(C):
                                for s4 in range(SB):
                                    nc.tensor.matmul(
                                        ps[s4][:],
                                        lhsT=w_q[c][:, 128 * mi : 128 * (mi + 1)],
                                        rhs=xt[(s4, c)][:],
                                        start=(c == 0), stop=(c == C - 1),
                                    )
                            for s4 in range(SB):
                                s = SB * sb + s4
                                o = pcp.tile([128, 512], BF, name="pcp_t")
                                nc.scalar.copy(o[:], ps[s4][:])
                                nc.gpsimd.dma_start(out=outT.ap()[m, s, :, :], in_=o[:])

            project_dmajor(wa, xqT, NQ, qT)   # q'^T = (x_q @ A)^T

        if variant == "proj":
            with tc.tile_pool(name="drain", bufs=4) as dp:
                z = dp.tile([128, 512], F32, name="drain_t")
                nc.vector.memset(z[:], 0.0)
                for i in range(4):
                    for ep in range(4):
                        for u in range(4):
                            nc.sync.dma_start(out=out.ap()[i, ep, u, :, :], in_=z[:])
            return

        # ---- Attention, strip-pair by strip-pair ----
        # Pair p covers strips 2p (queries [512p, 512p+256), key bound
        # 512(2p+1)) and 2p+1 (queries [512p+256, 512p+512), bound
        # 512(2p+2)). Scores run pair-wide (N=512) except the last key
        # group (odd member only, N=256). Z^T accumulates in 4 d-quarter
        # passes; the Wv projection then runs per 512-e pass.
        with (
            tc.tile_pool(name="ones", bufs=1) as onesp,
            tc.tile_pool(name="wv", bufs=1) as wvp,
            tc.tile_pool(name="pt", bufs=36) as ptp,
            tc.tile_pool(name="xg", bufs=8) as xgp,
            tc.tile_pool(name="zt", bufs=20) as ztp,
            tc.tile_pool(name="msk", bufs=4) as mskp,
            tc.tile_pool(name="dac", bufs=5) as dacp,
            tc.tile_pool(name="rcp", bufs=6) as rcpp,
            tc.tile_pool(name="osb", bufs=5) as osbp,
            tc.tile_pool(name="ps_s", bufs=2, space="PSUM") as ps_s,
            tc.tile_pool(name="ps_zo", bufs=4, space="PSUM") as ps_zo,
            tc.tile_pool(name="ps_d", bufs=1, space="PSUM") as ps_d,
        ):
            onesf = onesp.tile([128, 1], F32, name="onesf")
            nc.vector.memset(onesf[:], 1.0)
            idf = onesp.tile([1, 1], F32, name="idf")
            nc.vector.memset(idf[:], 1.0)

            wv_sb = []
            for c in range(C):
                t = wvp.tile([128, D], BF, name=f"wv_sb{c}")
                nc.gpsimd.dma_start(out=t[:], in_=wv.ap()[128 * c : 128 * (c + 1), :])
                wv_sb.append(t)

            NPAIR = STRIPS // 2
            for i in range(NPAIR):
                ng_even = 2 * i + 1   # groups for subs 0,1 (strip 2i)
                ng_odd = 2 * i + 2    # groups for subs 2,3 (strip 2i+1)
                qs = []
                for c in range(C):
                    t = qsp.tile([128, 512], BF, name="qs_t")
                    nc.sync.dma_start(out=t[:], in_=qT.ap()[c, i, :, :])
                    qs.append(t)

                # Phase A: P^T chunks. dacc[kk] accumulates the pt chunks
                # elementwise on DVE (f32) so the denominator needs only one
                # partition-reducing ones-matmul per pair instead of one per
                # chunk.
                pt = []
                co_of = {}
                dacc = [
                    dacp.tile([128, 512], F32, name="dac_t") for _ in range(4)
                ]
                for g in range(ng_odd):
                    full_pair = g < ng_even  # last group: odd member only
                    kg = []
                    for c in range(C):
                        t = kgp.tile([128, 512], BF, name="kg_t")
                        nc.sync.dma_start(out=t[:], in_=xkT.ap()[c, g, :, :])
                        kg.append(t)
                    for kk in range(4):
                        # column start: diagonal-group chunks whose low
                        # columns are fully causal-masked for BOTH halves
                        # are skipped (those pt regions are memset to 0).
                        if g == 2 * i:        # diag of strip 2i
                            co = 0 if kk < 2 else 128
                        elif not full_pair:   # last group: diag of 2i+1
                            co = 256 if kk < 2 else 384
                        else:
                            co = 0
                        ps = ps_s.tile([128, 512], F32, name="ps_s_t")
                        for c in range(C):
                            nc.tensor.matmul(
                                ps[:, co:512],
                                lhsT=kg[c][:, 128 * kk : 128 * (kk + 1)],
                                rhs=qs[c][:, co:512],
                                start=(c == 0), stop=(c == C - 1),
                            )
                        p = ptp.tile([128, 512], BF, name="pt_t")
                        if co > 0:
                            nc.vector.memset(p[:, 0:co], 0.0)
                        nc.scalar.activation(
                            out=p[:, co:512], in_=ps[:, co:512],
                            func=mybir.ActivationFunctionType.Exp, scale=SCALE,
                        )
                        co_of[4 * g + kk] = co
                        # diagonal-group masks, per member strip
                        for member, js in ((0, 2 * i), (1, 2 * i + 1)):
                            if g == js:
                                mk = mskp.tile([128, SQ], BF, name="msk_t")
                                r0 = 512 * js + 128 * kk
                                nc.sync.dma_start(
                                    out=mk[:], in_=maskT.ap()[r0 : r0 + 128, :]
                                )
                                cols = slice(256 * member, 256 * (member + 1))
                                nc.vector.tensor_mul(p[:, cols], p[:, cols], mk[:])
                        if g == 0:
                            nc.vector.tensor_copy(dacc[kk][:], p[:])
                        else:
                            nc.vector.tensor_add(dacc[kk][:], dacc[kk][:], p[:])
                        pt.append(p)

                # Phase B: denominators. dacc holds per-kk partial sums of
                # the pt chunks; combine on DVE, then ONE f32 ones-matmul
                # reduces the 128 key partitions, and 4 PE transposes turn
                # the [1,512] row into per-sub [128,1] scalars.
                nc.vector.tensor_add(dacc[0][:], dacc[0][:], dacc[1][:])
                nc.vector.tensor_add(dacc[2][:], dacc[2][:], dacc[3][:])
                nc.vector.tensor_add(dacc[0][:], dacc[0][:], dacc[2][:])
                dn1 = ps_d.tile([1, 512], F32, name="dn1_t")
                nc.tensor.matmul(
                    dn1[:], lhsT=onesf[:, 0:1], rhs=dacc[0][:],
                    start=True, stop=True,
                )
                dn_sb = rcpp.tile([1, 512], F32, name="dnsb_t")
                nc.scalar.copy(dn_sb[:], dn1[:])
                rec_sb = [None] * 4
                for u in range(4):
                    dt_ps = ps_d.tile([128, 1], F32, name="dnT_t")
                    nc.tensor.transpose(
                        dt_ps[:], dn_sb[0:1, 128 * u : 128 * (u + 1)],
                        idf[0:1, 0:1],
                    )
                    r = rcpp.tile([128, 1], F32, name="rec_t")
                    nc.vector.reciprocal(r[:], dt_ps[:])
                    rec_sb[u] = r

                # Phase B': Z^T = sum_k x[k,:]^T P^T[k,:] in 4 d-quarter
                # passes. For the last key group only query cols 256:512 are
                # valid, so those matmuls accumulate into the right half.
                zt = {}
                for qp in range(4):
                    z_ps = [
                        ps_zo.tile([128, 512], F32, name="zo_ps") for _ in range(4)
                    ]
                    for g in range(ng_odd):
                        for kk in range(4):
                            kc = 4 * g + kk
                            vt = xgp.tile([128, 512], BF, name="xg_t")
                            nc.scalar.dma_start(out=vt[:], in_=xn.ap()[qp, g, kk, :, :])
                            co = co_of[kc]
                            for c4 in range(4):
                                first = g == 0 and kk == 0
                                last = g == ng_odd - 1 and kk == 3
                                nc.tensor.matmul(
                                    z_ps[c4][:, co:512],
                                    lhsT=vt[:, 128 * c4 : 128 * (c4 + 1)],
                                    rhs=pt[kc][:, co:512],
                                    start=first, stop=last,
                                )
                    for c4 in range(4):
                        zt_t = ztp.tile([128, 512], BF, name="zt_t")
                        nc.scalar.copy(zt_t[:], z_ps[c4][:])
                        zt[4 * qp + c4] = zt_t

                # Phase C: out = Z @ Wv, then normalize by 1/den and store.
                for ep in range(4):
                    o_ps = [
                        ps_zo.tile([128, 512], F32, name="zo_ps") for _ in range(4)
                    ]
                    for u in range(4):
                        for c in range(C):
                            nc.tensor.matmul(
                                o_ps[u][:],
                                lhsT=zt[c][:, 128 * u : 128 * (u + 1)],
                                rhs=wv_sb[c][:, 512 * ep : 512 * (ep + 1)],
                                start=(c == 0), stop=(c == C - 1),
                            )
                    for u in range(4):
                        o = osbp.tile([128, 512], F32, name="osb_t")
                        nc.vector.tensor_scalar_mul(o[:], o_ps[u][:], rec_sb[u][:])
                        nc.gpsimd.dma_start(out=out.ap()[i, ep, u, :, :], in_=o[:])


_NC_CACHE = {}


def _get_nc(variant="full", reps=1):
    key = (variant, reps)
    if key not in _NC_CACHE:
        _NC_CACHE[key] = build_nc(variant, reps)
    return _NC_CACHE[key]


def _dmajor_tiles(xt: np.ndarray) -> np.ndarray:
    """[rows, D] -> [C, rows/512, 128, 512] (d-major, tile-contiguous)."""
    rows = xt.shape[0]
    return np.ascontiguousarray(
        xt.T.reshape(C, 128, rows // 512, 512).transpose(0, 2, 1, 3)
    )


def _core_inputs(x, A_t, Wv, b, h):
    blocks = _blocks_for_half(h)
    qpos = (128 * np.asarray(blocks)[:, None] + np.arange(128)[None, :]).reshape(-1)
    xb = np.asarray(x[b], dtype=ml_dtypes.bfloat16)
    xq = xb[qpos]
    maskT = np.zeros((S, SQ), dtype=np.float32)
    for j in range(STRIPS):
        keys = 512 * j + np.arange(512)[:, None]
        qp = qpos[SQ * j : SQ * (j + 1)][None, :]
        maskT[512 * j : 512 * (j + 1), :] = (keys <= qp).astype(np.float32)
    # xn tiled [qp, g, kk, 128, 512]: block (g,kk) rows, qp-th 512 d-cols
    xn_t = np.ascontiguousarray(
        xb.reshape(NG, 4, 128, 4, 512).transpose(3, 0, 1, 2, 4)
    )
    return {
        "xqT": _dmajor_tiles(xq),
        "xkT": _dmajor_tiles(xb),
        "xn": xn_t,
        "A": A_t,
        "Wv": np.ascontiguousarray(Wv).astype(ml_dtypes.bfloat16),
        "maskT": maskT.astype(ml_dtypes.bfloat16),
    }, qpos


def kernel(x, Wq, Wk, Wv, _want_results=False):
    x = np.asarray(x)
    Wq, Wk, Wv = np.asarray(Wq), np.asarray(Wk), np.asarray(Wv)
    A = (Wq.astype(np.float32) @ Wk.astype(np.float32).T).astype(ml_dtypes.bfloat16)
    # [qtr, c, 128, 512] tile-contiguous
    A_t = np.ascontiguousarray(A.reshape(C, 128, 4, 512).transpose(2, 0, 1, 3))
    nc = _get_nc()

    in_maps, qposes = [], []
    for b in range(B):
        for h in range(2):
            im, qpos = _core_inputs(x, A_t, Wv, b, h)
            in_maps.append(im)
            qposes.append((b, qpos))

    res = run_bass_kernel_spmd(nc, in_maps, core_ids=list(range(8)))

    out = np.empty((B, S, D), dtype=np.float32)
    for core, (b, qpos) in enumerate(qposes):
        # [i, ep, u, 128, 512] -> [i*4*128 (rows), ep*512 (cols)]
        o = res.results[core]["out"].transpose(0, 2, 3, 1, 4).reshape(NQ, D)
        out[b][qpos] = o
    if _want_results:
        return out, res
    return out


def measure_exec_ns(inputs, iters=48, variant="full"):
    """Estimate per-launch device execution time by pipelining `iters`
    dispatches of the compiled executable with device-resident inputs
    (amortizes host/tunnel dispatch overhead); returns marginal ns/exec."""
    import time
    import jax
    from jax.sharding import Mesh, PartitionSpec, NamedSharding
    from jax.experimental.shard_map import shard_map
    from concourse.bass2jax import (
        _bass_exec_p, install_neuronx_cc_hook, partition_id_tensor,
    )

    nc = _get_nc(variant)
    install_neuronx_cc_hook()
    in_names, out_names, out_avals, zero_outs = [], [], [], []
    for alloc in nc.m.functions[0].allocations:
        if not isinstance(alloc, mybir.MemoryLocationSet):
            continue
        name = alloc.memorylocations[0].name
        if alloc.kind == "ExternalInput":
            if nc.partition_id_tensor is None or name != nc.partition_id_tensor.name:
                in_names.append(name)
        elif alloc.kind == "ExternalOutput":
            out_names.append(name)
            shape = tuple(alloc.tensor_shape)
            dtype = mybir.dt.np(alloc.dtype)
            out_avals.append(jax.core.ShapedArray(shape, dtype))
            zero_outs.append(np.zeros(shape, dtype))
    n_params = len(in_names)
    n_outs = len(out_avals)
    all_names = in_names + out_names
    if nc.partition_id_tensor is not None:
        all_names = all_names + [nc.partition_id_tensor.name]

    def _body(*args):
        operands = list(args)
        if nc.partition_id_tensor is not None:
            operands.append(partition_id_tensor())
        return tuple(_bass_exec_p.bind(
            *operands, out_avals=tuple(out_avals), in_names=tuple(all_names),
            out_names=tuple(out_names), lowering_input_output_aliases=(),
            sim_require_finite=True, sim_require_nnan=True, nc=nc,
        ))

    devices = jax.devices()[:8]
    mesh = Mesh(np.array(devices), ("core",))
    sharded = jax.jit(
        shard_map(_body, mesh=mesh,
                  in_specs=(PartitionSpec("core"),) * (n_params + n_outs),
                  out_specs=(PartitionSpec("core"),) * n_outs,
                  check_rep=False),
        donate_argnums=tuple(range(n_params, n_params + n_outs)),
        keep_unused=True,
    )
    in_maps = []
    x, Wq, Wk, Wv = inputs["x"], inputs["Wq"], inputs["Wk"], inputs["Wv"]
    A = (np.asarray(Wq, np.float32) @ np.asarray(Wk, np.float32).T).astype(
        ml_dtypes.bfloat16
    )
    A_t = np.ascontiguousarray(A.reshape(C, 128, 4, 512).transpose(2, 0, 1, 3))
    for b in range(B):
        for h in range(2):
            im, _ = _core_inputs(x, A_t, Wv, b, h)
            in_maps.append(im)
    sh = NamedSharding(mesh, PartitionSpec("core"))
    concat_in = [
        jax.device_put(
            np.concatenate([np.asarray(in_maps[c][n]) for c in range(8)], axis=0), sh
        )
        for n in in_names
    ]

    def put_zeros():
        return [
            jax.device_put(np.zeros((8 * z.shape[0], *z.shape[1:]), z.dtype), sh)
            for z in zero_outs
        ]

    jax.block_until_ready(sharded(*concat_in, *put_zeros()))  # warmup
    times = {}
    for K in (4, iters, 4, iters):
        zs = [put_zeros() for _ in range(K)]
        jax.block_until_ready(zs)
        t0 = time.time()
        outs = [sharded(*concat_in, *z) for z in zs]
        jax.block_until_ready(outs)
        times[K] = min(times.get(K, 1e9), time.time() - t0)
    slope = (times[iters] - times[4]) / (iters - 4)
    return int(slope * 1e9)
